# revision 2
# baseline (speedup 1.0000x reference)
"""
Trainium2 Bass kernel for nn_MetaAttention.

Computation (per batch b):
    rowsum[h,i]     = sum_j m[b,h,i,j]
    aggregated[i,j] = sum_h rowsum[h,i] * m[b,h,i,j]
    out[b]          = softmax(aggregated.flatten()).reshape(N, N)

Sharding: pure data parallel over B=16 across 8 cores (2 batches/core).

Per-core strategy (memory regime, ~64 MB HBM traffic/core; the SDMA
engines cap out around 16 GB/s each under all-core load, so the kernel
is structured to keep the DMA stream busy end-to-end):
  - Row tiles of P=112 partitions; partition p holds CONTIGUOUS rows
    7p..7p+6 ("(p t) j") so DMA descriptors are 9-12 KB contiguous
    DRAM segments. Row permutation is transparent (math is
    row-independent; the store inverts the map).
  - Loads are per (batch, PSUM-chunk of 4/3 row tiles, head) so the
    pipeline stays fine-grained; deep mh buffering keeps DMA ahead.
  - The scale-accumulate over heads is split across engine paths (fp32
    PE matmul is 2-pass + half-rate, so PE alone can't carry it): per
    (head, tile) units go to PE diag-matmul (PSUM), DVE fused
    scalar_tensor_tensor (SBUF), or ACT-mult + GPSIMD-add (SBUF), with
    rotating assignments so each h-step has a near-constant mix. A DVE
    add merges PSUM partials into the SBUF agg.
  - rowsums: half the heads via one multi-tile DVE tensor_reduce per
    (chunk, head), half via per-tile ACT activation+accum_out.
  - Softmax: per-tile max (DVE), cross-partition max/sum via gpsimd
    partition_all_reduce, exp with fused sums (ACT), per-tile final
    scale (ACT) + per-tile store (SWDGE) to overlap the tail.
"""

import numpy as np

B, H, N = 16, 12, 784
NCORES = 8
BPC = B // NCORES          # batches per core
P = 112                    # partition tile (784 = 7 * 112)
NT = N // P                # 7 row tiles
CHUNKS = [(0, 3), (3, 4)]  # (first row-tile, n row-tiles) per PSUM chunk
JSPLITS = [(0, 512), (512, 272)]  # matmul free-dim splits (PSUM bank aligned)
ROWSUM_DVE_H = {0, 2, 4, 6, 8, 10}  # multi-tile DVE reduce; rest per-tile ACT

LAST_RESULT = None  # BassKernelResults of the most recent kernel() call


def unit_path(h, k, tail_chunk=False):
    """Engine path of the scale-accumulate unit (head h, row tile k):
    'pe' | 'init' | 'dve' | 'gps'. Normally 6 PE heads per tile,
    rotating by k; SBUF chain runs init -> dve -> dve -> gps -> gps ->
    gps (one cross-engine hop per tile). For the final chunk of the
    last batch (tail_chunk) the slow GPSIMD links are excluded so the
    post-last-load tail is short: 8 PE heads, init + 3 DVE.
    """
    npe = 8 if tail_chunk else 6
    if (h + 2 * k) % 12 < npe:
        return "pe"
    sbuf_heads = [hh for hh in range(H) if (hh + 2 * k) % 12 >= npe]
    idx = sbuf_heads.index(h)
    if idx == 0:
        return "init"
    if tail_chunk:
        return "dve"
    return "dve" if idx <= 2 else "gps"


def build_program():
    import concourse.bacc as bacc
    import concourse.tile as tile
    from concourse import mybir
    from concourse import bass_isa

    f32 = mybir.dt.float32
    nc = bacc.Bacc("TRN2")

    x = nc.dram_tensor("x", [BPC, H, N, N], f32, kind="ExternalInput")
    ident = nc.dram_tensor("ident", [P, P], f32, kind="ExternalInput")
    y = nc.dram_tensor("y", [BPC, N, N], f32, kind="ExternalOutput")

    with tile.TileContext(nc) as tc:
        with (
            tc.tile_pool(name="mh", bufs=7) as mh_pool,
            tc.tile_pool(name="agg", bufs=2) as agg_pool,
            tc.tile_pool(name="acc", bufs=4, space="PSUM") as acc_pool,
            tc.tile_pool(name="diag", bufs=4) as diag_pool,
            tc.tile_pool(name="scratch", bufs=4) as scratch_pool,
            tc.tile_pool(name="small", bufs=8) as small_pool,
            tc.tile_pool(name="consts", bufs=1) as const_pool,
        ):
            ident_sb = const_pool.tile([P, P], f32)
            nc.sync.dma_start(out=ident_sb, in_=ident[:, :])
            ones_sb = const_pool.tile([P, P], f32)
            nc.vector.memset(ones_sb, 1.0)

            for b in range(BPC):
                agg = agg_pool.tile([P, NT, N], f32, tag="agg")
                maxs = small_pool.tile([P, NT], f32, tag="maxs")
                sums = small_pool.tile([P, NT], f32, tag="sums")

                for ci, (c0, ct) in enumerate(CHUNKS):
                    tailc = (b == BPC - 1) and (ci == len(CHUNKS) - 1)
                    accs = [
                        acc_pool.tile([P, 1024], f32, tag="acc",
                                      name=f"acc_{b}_{c0}_{k}")
                        for k in range(ct)
                    ]
                    pe_first = {
                        k: min(h for h in range(H)
                               if unit_path(h, c0 + k, tailc) == "pe")
                        for k in range(ct)
                    }
                    pe_last = {
                        k: max(h for h in range(H)
                               if unit_path(h, c0 + k, tailc) == "pe")
                        for k in range(ct)
                    }
                    for h in range(H):
                        mh = mh_pool.tile([P, ct, N], f32, tag="mh")
                        # partition p <- contiguous rows 7p..7p+6 of m[b,h]
                        src = x[b, h].rearrange("(p t) j -> p t j", p=P)
                        # alternate the two HWDGE rings (SP / ACT) so each
                        # SDMA engine round-robins two packet streams
                        dma_eng = nc.sync if h % 2 == 0 else nc.scalar
                        dma_eng.dma_start(out=mh, in_=src[:, c0 : c0 + ct, :])

                        if h in ROWSUM_DVE_H:
                            rs7 = small_pool.tile([P, ct], f32, tag="rs7")
                            nc.vector.tensor_reduce(
                                out=rs7, in_=mh, axis=mybir.AxisListType.X,
                                op=mybir.AluOpType.add,
                            )
                            rs_of = lambda k: rs7[:, k : k + 1]
                        else:
                            rs7a = small_pool.tile([P, ct], f32, tag="rs7a")
                            scr = scratch_pool.tile([P, N], f32, tag="scr")
                            for k in range(ct):
                                nc.scalar.activation(
                                    out=scr, in_=mh[:, k, :],
                                    func=mybir.ActivationFunctionType.Copy,
                                    bias=0.0, scale=1.0,
                                    accum_out=rs7a[:, k : k + 1],
                                )
                            rs_of = lambda k: rs7a[:, k : k + 1]

                        for k in range(ct):
                            it = c0 + k
                            rs = rs_of(k)
                            p_ = unit_path(h, it, tailc)
                            if p_ == "pe":
                                dg = diag_pool.tile([P, P], f32, tag="dg")
                                nc.vector.tensor_scalar_mul(
                                    out=dg, in0=ident_sb, scalar1=rs
                                )
                                for j0, jn in JSPLITS:
                                    nc.tensor.matmul(
                                        accs[k][:, j0 : j0 + jn],
                                        lhsT=dg,
                                        rhs=mh[:, k, j0 : j0 + jn],
                                        start=(h == pe_first[k]),
                                        stop=(h == pe_last[k]),
                                    )
                            elif p_ == "init":
                                nc.vector.tensor_scalar_mul(
                                    out=agg[:, it, :], in0=mh[:, k, :], scalar1=rs
                                )
                            elif p_ == "dve":
                                nc.vector.scalar_tensor_tensor(
                                    out=agg[:, it, :],
                                    in0=mh[:, k, :],
                                    scalar=rs,
                                    in1=agg[:, it, :],
                                    op0=mybir.AluOpType.mult,
                                    op1=mybir.AluOpType.add,
                                )
                            else:  # gps: scale on ACT, add on gpsimd
                                sc2 = scratch_pool.tile([P, N], f32, tag="sc2")
                                nc.scalar.activation(
                                    out=sc2, in_=mh[:, k, :],
                                    func=mybir.ActivationFunctionType.Copy,
                                    bias=0.0, scale=rs,
                                )
                                nc.gpsimd.tensor_tensor(
                                    out=agg[:, it, :],
                                    in0=sc2,
                                    in1=agg[:, it, :],
                                    op=mybir.AluOpType.add,
                                )
                    # merge PSUM partial into agg; per-tile max
                    for k in range(ct):
                        it = c0 + k
                        nc.vector.tensor_add(
                            out=agg[:, it, :],
                            in0=agg[:, it, :],
                            in1=accs[k][:, 0:N],
                        )
                        nc.vector.tensor_reduce(
                            out=maxs[:, it : it + 1],
                            in_=agg[:, it, :],
                            axis=mybir.AxisListType.X,
                            op=mybir.AluOpType.max,
                        )

                # ---- softmax over the full [N, N] of this batch ----
                m1 = small_pool.tile([P, 1], f32, tag="m1")
                nc.vector.tensor_reduce(
                    out=m1, in_=maxs, axis=mybir.AxisListType.X,
                    op=mybir.AluOpType.max,
                )
                # cross-partition max: PE transpose -> free-axis reduce ->
                # K=1 all-ones matmul broadcast (low latency; gpsimd
                # partition_all_reduce costs ~5us of Q7 dispatch)
                tps = acc_pool.tile([1, P], f32, tag="acc", name=f"tps_{b}")
                nc.tensor.transpose(tps, m1, ident_sb)
                gm = small_pool.tile([1, 1], f32, tag="gm")
                nc.vector.tensor_reduce(
                    out=gm, in_=tps, axis=mybir.AxisListType.X,
                    op=mybir.AluOpType.max,
                )
                bps = acc_pool.tile([P, 1], f32, tag="acc", name=f"bps_{b}")
                nc.tensor.matmul(bps, lhsT=ones_sb[0:1, :], rhs=gm,
                                 start=True, stop=True)
                negmax = small_pool.tile([P, 1], f32, tag="negmax")
                nc.scalar.mul(out=negmax, in_=bps, mul=-1.0)

                for it in range(NT):
                    nc.scalar.activation(
                        out=agg[:, it, :],
                        in_=agg[:, it, :],
                        func=mybir.ActivationFunctionType.Exp,
                        bias=negmax,
                        scale=1.0,
                        accum_out=sums[:, it : it + 1],
                    )
                s1 = small_pool.tile([P, 1], f32, tag="s1")
                nc.vector.tensor_reduce(
                    out=s1, in_=sums, axis=mybir.AxisListType.X,
                    op=mybir.AluOpType.add,
                )
                # cross-partition sum + broadcast in one all-ones matmul
                sps = acc_pool.tile([P, 1], f32, tag="acc", name=f"sps_{b}")
                nc.tensor.matmul(sps, lhsT=ones_sb, rhs=s1, start=True, stop=True)
                rinv = small_pool.tile([P, 1], f32, tag="rinv")
                nc.vector.reciprocal(out=rinv, in_=sps)

                # per-tile scale + store so the tail pipelines; alternate
                # the scale between ACT and DVE to halve its serial latency
                dst = y[b].rearrange("(p t) j -> p t j", p=P)
                for it in range(NT):
                    if it % 2 == 0:
                        nc.scalar.activation(
                            out=agg[:, it, :],
                            in_=agg[:, it, :],
                            func=mybir.ActivationFunctionType.Copy,
                            bias=0.0,
                            scale=rinv,
                        )
                    else:
                        nc.vector.tensor_scalar_mul(
                            out=agg[:, it, :], in0=agg[:, it, :], scalar1=rinv
                        )
                    nc.gpsimd.dma_start(
                        out=dst[:, it, :], in_=agg[:, it, :]
                    )

    nc.finalize()  # Bacc: register alloc, nop/event-sem legalization, ISA codegen
    return nc


def kernel(mha_masks) -> np.ndarray:
    global LAST_RESULT
    from concourse.bass_utils import run_bass_kernel_spmd

    xfull = np.ascontiguousarray(np.asarray(mha_masks, dtype=np.float32))
    assert xfull.shape == (B, H, N, N), xfull.shape

    nc = build_program()
    ident = np.eye(P, dtype=np.float32)
    in_maps = [
        {"x": xfull[i * BPC : (i + 1) * BPC], "ident": ident}
        for i in range(NCORES)
    ]
    import os

    kw = {}
    if os.environ.get("KERNEL_TRACE_DIR"):
        kw = dict(trace=True, tmpdir=os.environ["KERNEL_TRACE_DIR"])
    res = run_bass_kernel_spmd(nc, in_maps, core_ids=list(range(NCORES)), **kw)
    LAST_RESULT = res
    out = np.concatenate(
        [np.asarray(r["y"], dtype=np.float32) for r in res.results], axis=0
    )
    return out



# revision 4
# speedup vs baseline: 1.0269x; 1.0269x over previous
"""
Trainium2 Bass kernel for nn_MetaAttention.

Computation (per batch b):
    rowsum[h,i]     = sum_j m[b,h,i,j]
    aggregated[i,j] = sum_h rowsum[h,i] * m[b,h,i,j]
    out[b]          = softmax(aggregated.flatten()).reshape(N, N)

Sharding: pure data parallel over B=16 across 8 cores (2 batches/core).

Per-core strategy (memory regime, ~64 MB HBM traffic/core; the SDMA
engines cap out around 16 GB/s each under all-core load, so the kernel
is structured to keep the DMA stream busy end-to-end):
  - Row tiles of P=112 partitions; partition p holds CONTIGUOUS rows
    7p..7p+6 ("(p t) j") so DMA descriptors are 9-12 KB contiguous
    DRAM segments. Row permutation is transparent (math is
    row-independent; the store inverts the map).
  - Loads are per (batch, PSUM-chunk of 4/3 row tiles, head) so the
    pipeline stays fine-grained; deep mh buffering keeps DMA ahead.
  - The scale-accumulate over heads is split across engine paths (fp32
    PE matmul is 2-pass + half-rate, so PE alone can't carry it): per
    (head, tile) units go to PE diag-matmul (PSUM), DVE fused
    scalar_tensor_tensor (SBUF), or ACT-mult + GPSIMD-add (SBUF), with
    rotating assignments so each h-step has a near-constant mix. A DVE
    add merges PSUM partials into the SBUF agg.
  - rowsums: half the heads via one multi-tile DVE tensor_reduce per
    (chunk, head), half via per-tile ACT activation+accum_out.
  - Softmax: per-tile max (DVE), cross-partition max/sum via gpsimd
    partition_all_reduce, exp with fused sums (ACT), per-tile final
    scale (ACT) + per-tile store (SWDGE) to overlap the tail.
"""

import numpy as np

B, H, N = 16, 12, 784
NCORES = 8
BPC = B // NCORES          # batches per core
P = 112                    # partition tile (784 = 7 * 112)
NT = N // P                # 7 row tiles
CHUNKS = [(0, 3), (3, 4)]  # (first row-tile, n row-tiles) per PSUM chunk
JSPLITS = [(0, 512), (512, 272)]  # matmul free-dim splits (PSUM bank aligned)
ROWSUM_DVE_H = {0, 2, 4, 6, 8, 10}  # multi-tile DVE reduce; rest per-tile ACT

LAST_RESULT = None  # BassKernelResults of the most recent kernel() call


def unit_path(h, k, tail_chunk=False):
    """Engine path of the scale-accumulate unit (head h, row tile k):
    'pe' | 'init' | 'dve' | 'gps'. Normally 6 PE heads per tile,
    rotating by k; SBUF chain runs init -> dve -> dve -> gps -> gps ->
    gps (one cross-engine hop per tile). For the final chunk of the
    last batch (tail_chunk) the slow GPSIMD links are excluded so the
    post-last-load tail is short: 8 PE heads, init + 3 DVE.
    """
    npe = 8 if tail_chunk else 6
    if (h + 2 * k) % 12 < npe:
        return "pe"
    sbuf_heads = [hh for hh in range(H) if (hh + 2 * k) % 12 >= npe]
    idx = sbuf_heads.index(h)
    if idx == 0:
        return "init"
    if tail_chunk:
        return "dve"
    return "dve" if idx <= 2 else "gps"


def build_program():
    import concourse.bacc as bacc
    import concourse.tile as tile
    from concourse import mybir
    from concourse import bass_isa

    f32 = mybir.dt.float32
    nc = bacc.Bacc("TRN2")

    x = nc.dram_tensor("x", [BPC, H, N, N], f32, kind="ExternalInput")
    ident = nc.dram_tensor("ident", [P, P], f32, kind="ExternalInput")
    y = nc.dram_tensor("y", [BPC, N, N], f32, kind="ExternalOutput")

    with tile.TileContext(nc) as tc:
        with (
            tc.tile_pool(name="mh", bufs=7) as mh_pool,
            tc.tile_pool(name="agg", bufs=2) as agg_pool,
            tc.tile_pool(name="acc", bufs=4, space="PSUM") as acc_pool,
            tc.tile_pool(name="diag", bufs=4) as diag_pool,
            tc.tile_pool(name="scratch", bufs=4) as scratch_pool,
            tc.tile_pool(name="small", bufs=8) as small_pool,
            tc.tile_pool(name="consts", bufs=1) as const_pool,
        ):
            ident_sb = const_pool.tile([P, P], f32)
            nc.sync.dma_start(out=ident_sb, in_=ident[:, :])
            ones_sb = const_pool.tile([P, P], f32)
            nc.vector.memset(ones_sb, 1.0)

            for b in range(BPC):
                agg = agg_pool.tile([P, NT, N], f32, tag="agg")
                maxs = small_pool.tile([P, NT], f32, tag="maxs")
                sums = small_pool.tile([P, NT], f32, tag="sums")

                for ci, (c0, ct) in enumerate(CHUNKS):
                    tailc = (b == BPC - 1) and (ci == len(CHUNKS) - 1)
                    accs = [
                        acc_pool.tile([P, 1024], f32, tag="acc",
                                      name=f"acc_{b}_{c0}_{k}")
                        for k in range(ct)
                    ]
                    pe_first = {
                        k: min(h for h in range(H)
                               if unit_path(h, c0 + k, tailc) == "pe")
                        for k in range(ct)
                    }
                    pe_last = {
                        k: max(h for h in range(H)
                               if unit_path(h, c0 + k, tailc) == "pe")
                        for k in range(ct)
                    }
                    for h in range(H):
                        mh = mh_pool.tile([P, ct, N], f32, tag="mh")
                        # partition p <- contiguous rows 7p..7p+6 of m[b,h]
                        src = x[b, h].rearrange("(p t) j -> p t j", p=P)
                        nc.gpsimd.dma_start(out=mh, in_=src[:, c0 : c0 + ct, :])

                        if h in ROWSUM_DVE_H:
                            rs7 = small_pool.tile([P, ct], f32, tag="rs7")
                            nc.vector.tensor_reduce(
                                out=rs7, in_=mh, axis=mybir.AxisListType.X,
                                op=mybir.AluOpType.add,
                            )
                            rs_of = lambda k: rs7[:, k : k + 1]
                        else:
                            rs7a = small_pool.tile([P, ct], f32, tag="rs7a")
                            scr = scratch_pool.tile([P, N], f32, tag="scr")
                            for k in range(ct):
                                nc.scalar.activation(
                                    out=scr, in_=mh[:, k, :],
                                    func=mybir.ActivationFunctionType.Copy,
                                    bias=0.0, scale=1.0,
                                    accum_out=rs7a[:, k : k + 1],
                                )
                            rs_of = lambda k: rs7a[:, k : k + 1]

                        for k in range(ct):
                            it = c0 + k
                            rs = rs_of(k)
                            p_ = unit_path(h, it, tailc)
                            if p_ == "pe":
                                dg = diag_pool.tile([P, P], f32, tag="dg")
                                nc.vector.tensor_scalar_mul(
                                    out=dg, in0=ident_sb, scalar1=rs
                                )
                                for j0, jn in JSPLITS:
                                    nc.tensor.matmul(
                                        accs[k][:, j0 : j0 + jn],
                                        lhsT=dg,
                                        rhs=mh[:, k, j0 : j0 + jn],
                                        start=(h == pe_first[k]),
                                        stop=(h == pe_last[k]),
                                    )
                            elif p_ == "init":
                                nc.vector.tensor_scalar_mul(
                                    out=agg[:, it, :], in0=mh[:, k, :], scalar1=rs
                                )
                            elif p_ == "dve":
                                nc.vector.scalar_tensor_tensor(
                                    out=agg[:, it, :],
                                    in0=mh[:, k, :],
                                    scalar=rs,
                                    in1=agg[:, it, :],
                                    op0=mybir.AluOpType.mult,
                                    op1=mybir.AluOpType.add,
                                )
                            else:  # gps: scale on ACT, add on gpsimd
                                sc2 = scratch_pool.tile([P, N], f32, tag="sc2")
                                nc.scalar.activation(
                                    out=sc2, in_=mh[:, k, :],
                                    func=mybir.ActivationFunctionType.Copy,
                                    bias=0.0, scale=rs,
                                )
                                nc.gpsimd.tensor_tensor(
                                    out=agg[:, it, :],
                                    in0=sc2,
                                    in1=agg[:, it, :],
                                    op=mybir.AluOpType.add,
                                )
                    # merge PSUM partial into agg; per-tile max
                    for k in range(ct):
                        it = c0 + k
                        nc.vector.tensor_add(
                            out=agg[:, it, :],
                            in0=agg[:, it, :],
                            in1=accs[k][:, 0:N],
                        )
                        nc.vector.tensor_reduce(
                            out=maxs[:, it : it + 1],
                            in_=agg[:, it, :],
                            axis=mybir.AxisListType.X,
                            op=mybir.AluOpType.max,
                        )

                # ---- softmax over the full [N, N] of this batch ----
                m1 = small_pool.tile([P, 1], f32, tag="m1")
                nc.vector.tensor_reduce(
                    out=m1, in_=maxs, axis=mybir.AxisListType.X,
                    op=mybir.AluOpType.max,
                )
                # cross-partition max: PE transpose -> free-axis reduce ->
                # K=1 all-ones matmul broadcast (low latency; gpsimd
                # partition_all_reduce costs ~5us of Q7 dispatch)
                tps = acc_pool.tile([1, P], f32, tag="acc", name=f"tps_{b}")
                nc.tensor.transpose(tps, m1, ident_sb)
                gm = small_pool.tile([1, 1], f32, tag="gm")
                nc.vector.tensor_reduce(
                    out=gm, in_=tps, axis=mybir.AxisListType.X,
                    op=mybir.AluOpType.max,
                )
                bps = acc_pool.tile([P, 1], f32, tag="acc", name=f"bps_{b}")
                nc.tensor.matmul(bps, lhsT=ones_sb[0:1, :], rhs=gm,
                                 start=True, stop=True)
                negmax = small_pool.tile([P, 1], f32, tag="negmax")
                nc.scalar.mul(out=negmax, in_=bps, mul=-1.0)

                for it in range(NT):
                    nc.scalar.activation(
                        out=agg[:, it, :],
                        in_=agg[:, it, :],
                        func=mybir.ActivationFunctionType.Exp,
                        bias=negmax,
                        scale=1.0,
                        accum_out=sums[:, it : it + 1],
                    )
                s1 = small_pool.tile([P, 1], f32, tag="s1")
                nc.vector.tensor_reduce(
                    out=s1, in_=sums, axis=mybir.AxisListType.X,
                    op=mybir.AluOpType.add,
                )
                # cross-partition sum + broadcast in one all-ones matmul
                sps = acc_pool.tile([P, 1], f32, tag="acc", name=f"sps_{b}")
                nc.tensor.matmul(sps, lhsT=ones_sb, rhs=s1, start=True, stop=True)
                rinv = small_pool.tile([P, 1], f32, tag="rinv")
                nc.vector.reciprocal(out=rinv, in_=sps)

                # per-tile scale + store so the tail pipelines; alternate
                # the scale between ACT and DVE to halve its serial latency
                dst = y[b].rearrange("(p t) j -> p t j", p=P)
                for it in range(NT):
                    if it % 2 == 0:
                        nc.scalar.activation(
                            out=agg[:, it, :],
                            in_=agg[:, it, :],
                            func=mybir.ActivationFunctionType.Copy,
                            bias=0.0,
                            scale=rinv,
                        )
                    else:
                        nc.vector.tensor_scalar_mul(
                            out=agg[:, it, :], in0=agg[:, it, :], scalar1=rinv
                        )
                    nc.sync.dma_start(
                        out=dst[:, it, :], in_=agg[:, it, :]
                    )

    nc.finalize()  # Bacc: register alloc, nop/event-sem legalization, ISA codegen
    return nc


def kernel(mha_masks) -> np.ndarray:
    global LAST_RESULT
    from concourse.bass_utils import run_bass_kernel_spmd

    xfull = np.ascontiguousarray(np.asarray(mha_masks, dtype=np.float32))
    assert xfull.shape == (B, H, N, N), xfull.shape

    nc = build_program()
    ident = np.eye(P, dtype=np.float32)
    in_maps = [
        {"x": xfull[i * BPC : (i + 1) * BPC], "ident": ident}
        for i in range(NCORES)
    ]
    import os

    kw = {}
    if os.environ.get("KERNEL_TRACE_DIR"):
        kw = dict(trace=True, tmpdir=os.environ["KERNEL_TRACE_DIR"])
    res = run_bass_kernel_spmd(nc, in_maps, core_ids=list(range(NCORES)), **kw)
    LAST_RESULT = res
    out = np.concatenate(
        [np.asarray(r["y"], dtype=np.float32) for r in res.results], axis=0
    )
    return out

